# revision 2
# baseline (speedup 1.0000x reference)
"""Trainium2 Bass kernel for im2col Conv2d dot-product:
out[b, n] = <enc_x[b, n, :], w_flat> + bias.

Data-parallel over batch: 8 batches per NeuronCore x 8 cores.
Per core: x is [401408, 49] fp32 (~78.7 MB) -> out [401408] fp32.
Memory-bound: HBM roofline ~220 us/core at ~358 GB/s.

Single-pass compute via DVE tensor_tensor_scan (the fused multiply+reduce):
  state_t = g_t * state_{t-1} + x_t          (one scan op per tile)
with g the periodic-49 gate/ratio pattern
  g_0 = 0 (resets the recurrence at each window boundary)
  g_u = w[u-1] / w[u]   for u = 1..48,
so the state at the last position of window j is  <x_j, w> / w[48].
Proof: s_48 = sum_t x_t * prod_{u=t+1..48} g_u = sum_t x_t * w_t / w_48,
and the t=0 gate zeroes any carry from the previous window.

The scan is the ONLY whole-data compute pass (1 elem/cycle on DVE ~161us
total), so DVE stays under the 220us DMA floor.  ScalarE then extracts
every 49th lane-element with  out = Identity(state * w_48 + bias)  (a
[128, W] strided activation, ~0.3us/tile) and its DMA ring (HWDGE-ACT)
writes results out, decoupled from the input ring (HWDGE-SP).  GpSimd
only broadcasts the two constant tensors via SWDGE.
"""

from contextlib import ExitStack

import numpy as np

import concourse.bass as bass
import concourse.tile as tile
from concourse import mybir

B = 64
WINDOWS = 50176
K = 49
NCORES = 8
BPC = B // NCORES            # batches per core
NWIN = BPC * WINDOWS         # 401408 windows per core
P = 128                      # partitions

WBIG = 196                   # windows per partition, big tiles
WSMALL = 49                  # windows per partition, tail tiles
TBIG = 15
TSMALL = 4
assert TBIG * P * WBIG + TSMALL * P * WSMALL == NWIN

FP32 = mybir.dt.float32

_NC = None


def _build_nc():
    nc = bass.Bass(trn_type="TRN2", debug=False, num_devices=NCORES)

    x = nc.dram_tensor("x", [NWIN, K], FP32, kind="ExternalInput").ap()
    g = nc.dram_tensor("g", [WBIG * K], FP32, kind="ExternalInput").ap()
    wlast = nc.dram_tensor("wlast", [1], FP32, kind="ExternalInput").ap()
    b = nc.dram_tensor("b", [1], FP32, kind="ExternalInput").ap()
    out = nc.dram_tensor("out", [NWIN], FP32, kind="ExternalOutput").ap()

    mult = mybir.AluOpType.mult
    add = mybir.AluOpType.add

    with tile.TileContext(nc) as tc, ExitStack() as ctx:
        consts = ctx.enter_context(tc.tile_pool(name="consts", bufs=1))
        xpool = ctx.enter_context(tc.tile_pool(name="x", bufs=3))
        opool = ctx.enter_context(tc.tile_pool(name="o", bufs=4))

        # bias and w[48] broadcast to all partitions: [P, 1] each
        bb = consts.tile([P, 1], FP32)
        nc.gpsimd.dma_start(
            out=bb[:],
            in_=bass.AP(tensor=b.tensor, offset=b.offset, ap=[[0, P]] + list(b.ap)),
        )
        wl = consts.tile([P, 1], FP32)
        nc.gpsimd.dma_start(
            out=wl[:],
            in_=bass.AP(
                tensor=wlast.tensor, offset=wlast.offset, ap=[[0, P]] + list(wlast.ap)
            ),
        )
        # gate/ratio pattern broadcast to all partitions: [P, WBIG*K]
        gt = consts.tile([P, WBIG * K], FP32)
        nc.gpsimd.dma_start(
            out=gt[:],
            in_=bass.AP(tensor=g.tensor, offset=g.offset, ap=[[0, P]] + list(g.ap)),
        )
        gt_ap = gt[:]

        def do_tile(win_base, wn, name):
            fd = wn * K
            xt = xpool.tile([P, fd], FP32, tag="xt", name=f"xt{name}")
            xt_ap = xt[:]
            # partition p <- windows [win_base + p*wn, win_base + (p+1)*wn)
            src = bass.AP(
                tensor=x.tensor,
                offset=x.offset + win_base * K,
                ap=[[fd, P], [1, fd]],
            )
            nc.sync.dma_start(out=xt_ap, in_=src)

            # fused multiply+segmented-reduce: one in-place scan pass
            nc.vector.tensor_tensor_scan(
                out=xt_ap,
                data0=bass.AP(
                    tensor=gt_ap.tensor, offset=gt_ap.offset,
                    ap=[gt_ap.ap[0], [1, fd]],
                ),
                data1=xt_ap,
                initial=0.0,
                op0=mult,
                op1=add,
            )

            # extract lane 48 of every window: out = state * w48 + bias
            acc = opool.tile([P, wn], FP32, tag="acc", name=f"acc{name}")
            ext = bass.AP(
                tensor=xt_ap.tensor,
                offset=xt_ap.offset + 48,
                ap=[xt_ap.ap[0], [K, wn]],
            )
            nc.scalar.activation(
                out=acc[:], in_=ext,
                func=mybir.ActivationFunctionType.Identity,
                bias=bb[:, 0:1], scale=wl[:, 0:1],
            )
            dst = bass.AP(
                tensor=out.tensor,
                offset=out.offset + win_base,
                ap=[[wn, P], [1, wn]],
            )
            nc.scalar.dma_start(out=dst, in_=acc[:])

        base = 0
        for t in range(TBIG):
            do_tile(base, WBIG, f"b{t}")
            base += P * WBIG
        for t in range(TSMALL):
            do_tile(base, WSMALL, f"s{t}")
            base += P * WSMALL
        assert base == NWIN

    return nc


def _split_ctrl_waits(nc, max_waits=1):
    """Work around a walrus codegen limit on this build: instructions accept
    only one sync-wait command. Hoist extra waits onto dedicated no-op
    instructions inserted just before, preserving per-engine order."""
    from concourse import mybir

    for f in nc.m.functions:
        for blk in f.blocks:
            insts = blk.instructions
            i = 0
            while i < len(insts):
                ins = insts[i]
                if (
                    ins.sync_info is not None
                    and len(ins.sync_info.on_wait) > max_waits
                ):
                    waits = list(ins.sync_info.on_wait)
                    keep, extra = waits[:max_waits], waits[max_waits:]
                    ins.sync_info.on_wait = keep
                    for j, wchunk in enumerate(extra):
                        nop = mybir.InstNoOp(
                            name=f"{ins.name}-wsplit{j}",
                            sync_info=mybir.SyncInfo(on_wait=[wchunk], on_update=[]),
                            bass_nofuse=True,
                            engine=ins.engine,
                        )
                        nc.register_instruction(nop, overwrite=True)
                        insts.insert(i, nop)
                        i += 1
                i += 1


def _get_nc():
    global _NC
    if _NC is None:
        _NC = _build_nc()
        _split_ctrl_waits(_NC)
    return _NC


def run(enc_x, weight, bias, trace=False, **spmd_kwargs):
    """Run on 8 NeuronCores; returns (out [B, WINDOWS] fp32, BassKernelResults)."""
    from concourse.bass_utils import run_bass_kernel_spmd

    nc = _get_nc()
    xf = np.ascontiguousarray(np.asarray(enc_x), dtype=np.float32).reshape(
        NCORES, NWIN, K
    )
    wf = np.asarray(weight, dtype=np.float64).reshape(K)
    bf = np.ascontiguousarray(np.asarray(bias), dtype=np.float32).reshape(1)
    # gate/ratio pattern: g_0 = 0 resets each window; g_u = w[u-1]/w[u]
    r = np.zeros(K, dtype=np.float64)
    r[1:] = wf[:-1] / wf[1:]
    gf = np.ascontiguousarray(np.tile(r, WBIG), dtype=np.float32)
    wlf = np.ascontiguousarray(wf[-1:], dtype=np.float32)
    in_maps = [
        {"x": xf[i], "g": gf, "wlast": wlf, "b": bf} for i in range(NCORES)
    ]
    res = run_bass_kernel_spmd(
        nc, in_maps, list(range(NCORES)), trace=trace, **spmd_kwargs
    )
    out = np.stack([res.results[i]["out"] for i in range(NCORES)], axis=0)
    return out.reshape(B, WINDOWS), res


def kernel(enc_x, weight, bias, windows_nb=None):
    out, _ = run(enc_x, weight, bias)
    return out


# revision 4
# speedup vs baseline: 1.2273x; 1.2273x over previous
"""Trainium2 Bass kernel for im2col Conv2d dot-product (v4, bf16 hybrid):
out[b, n] = <enc_x[b, n, :], w_flat> + bias.

Data-parallel over batch: 8 batches per NeuronCore x 8 cores.
Per core x [401408, 49] fp32 (~78.7 MB); HBM roofline ~225 us/core.

Key moves (probe-measured rates per [128, 196*49] tile):
  - SWDGE cast-DMA loads x as bf16 (HBM still reads fp32 -> same 220us
    DMA floor, but halves SBUF traffic/footprint; the Pool-queue trigger
    costs only ~0.8us/tile and GpSimd does no other work -> no POOL-port
    contention with DVE).
  - multiply: DVE bf16 tensor_tensor runs in 2x mode (5.26us vs 10.1
    fp32); ~4.5 tile-equivalents go to the otherwise-idle ScalarE as 49
    strided per-k activation muls (0.72us/col, own SBUF ports).
  - segmented reduce: DVE tensor_reduce (10.1us, the only free-axis
    reducer).  DVE total ~177us, ACT ~175us, both under the DMA floor.
  - ScalarE-tile reduces are emitted 2 tiles late so the in-order DVE
    queue never stalls waiting on ACT's 35us multiply chain.
  - output DMAs ride the (otherwise idle) SP HWDGE ring; bias is added
    on the host (elementwise on the small gathered output).
bf16 rounding of x and w gives rel err ~2e-3, well inside the 2e-2 gate.
"""

from contextlib import ExitStack

import numpy as np

import concourse.bass as bass
import concourse.tile as tile
from concourse import mybir

B = 64
WINDOWS = 50176
K = 49
NCORES = 8
BPC = B // NCORES
NWIN = BPC * WINDOWS
P = 128

WBIG = 196
WSMALL = 49
TBIG = 15
TSMALL = 4
assert TBIG * P * WBIG + TSMALL * P * WSMALL == NWIN

# Mode per tile (15 big then 4 small): "dve" or "act" multiply.
MODES = ["dve", "dve", "act", "dve", "dve", "act", "dve", "dve", "act",
         "dve", "dve", "act", "dve", "act", "dve",
         "act", "dve", "act", "dve"]
ACT_LAG = 2   # act-tile reduce/out deferred this many tiles

FP32 = mybir.dt.float32
BF16 = mybir.dt.bfloat16

_NC = None


def _build_nc():
    nc = bass.Bass(trn_type="TRN2", debug=False, num_devices=NCORES)

    x = nc.dram_tensor("x", [NWIN, K], FP32, kind="ExternalInput").ap()
    w = nc.dram_tensor("w", [K], FP32, kind="ExternalInput").ap()
    out = nc.dram_tensor("out", [NWIN], FP32, kind="ExternalOutput").ap()

    mult = mybir.AluOpType.mult
    add = mybir.AluOpType.add

    with tile.TileContext(nc) as tc, ExitStack() as ctx:
        consts = ctx.enter_context(tc.tile_pool(name="consts", bufs=1))
        xpool = ctx.enter_context(tc.tile_pool(name="x", bufs=6))
        opool = ctx.enter_context(tc.tile_pool(name="o", bufs=8))

        wb = consts.tile([P, K], FP32)
        nc.gpsimd.dma_start(
            out=wb[:],
            in_=bass.AP(tensor=w.tensor, offset=w.offset, ap=[[0, P]] + list(w.ap)),
        )
        wbb = consts.tile([P, K], BF16)
        nc.vector.tensor_copy(out=wbb[:], in_=wb[:])
        wb_ap = wb[:]
        wbb_ap = wbb[:]

        def w_bcast(wn):
            return bass.AP(
                tensor=wbb_ap.tensor, offset=wbb_ap.offset,
                ap=[wbb_ap.ap[0], [0, wn], wbb_ap.ap[1]],
            )

        deferred = []   # [(emit_after_tile_idx, fn), ...]

        def finish_tile(xt_ap, wn, win_base, name):
            xt3d = bass.AP(tensor=xt_ap.tensor, offset=xt_ap.offset,
                           ap=[xt_ap.ap[0], [K, wn], [1, K]])
            acc = opool.tile([P, wn], FP32, tag="acc", name=f"acc{name}")
            nc.vector.tensor_reduce(out=acc[:], in_=xt3d,
                                    axis=mybir.AxisListType.X, op=add)
            dst = bass.AP(
                tensor=out.tensor, offset=out.offset + win_base,
                ap=[[wn, P], [1, wn]],
            )
            nc.sync.dma_start(out=dst, in_=acc[:])

        def do_tile(idx, win_base, wn, mode, name):
            fd = wn * K
            xt = xpool.tile([P, fd], BF16, tag="xt", name=f"xt{name}")
            xt_ap = xt[:]
            src = bass.AP(
                tensor=x.tensor,
                offset=x.offset + win_base * K,
                ap=[[fd, P], [1, fd]],
            )
            nc.gpsimd.dma_start(out=xt_ap, in_=src)   # SWDGE cast fp32->bf16

            if mode == "dve":
                xt3d = bass.AP(tensor=xt_ap.tensor, offset=xt_ap.offset,
                               ap=[xt_ap.ap[0], [K, wn], [1, K]])
                nc.vector.tensor_tensor(out=xt3d, in0=xt3d,
                                        in1=w_bcast(wn), op=mult)
                finish_tile(xt_ap, wn, win_base, name)
            else:
                for kk in range(K):
                    col = bass.AP(
                        tensor=xt_ap.tensor, offset=xt_ap.offset + kk,
                        ap=[xt_ap.ap[0], [K, wn]],
                    )
                    nc.scalar.activation(
                        out=col, in_=col,
                        func=mybir.ActivationFunctionType.Identity,
                        scale=wb_ap[:, kk:kk + 1],
                    )
                deferred.append(
                    (idx + ACT_LAG,
                     lambda a=xt_ap, b=wn, c=win_base, d=name: finish_tile(a, b, c, d))
                )

        def flush_deferred(now):
            while deferred and deferred[0][0] <= now:
                deferred.pop(0)[1]()

        base = 0
        for t in range(TBIG + TSMALL):
            wn = WBIG if t < TBIG else WSMALL
            do_tile(t, base, wn, MODES[t], f"t{t}")
            flush_deferred(t)
            base += P * wn
        flush_deferred(10**9)
        assert base == NWIN

    return nc


def _split_ctrl_waits(nc, max_waits=1):
    """Work around a walrus codegen limit on this build: instructions accept
    only one sync-wait command. Hoist extra waits onto dedicated no-op
    instructions inserted just before, preserving per-engine order."""
    from concourse import mybir

    for f in nc.m.functions:
        for blk in f.blocks:
            insts = blk.instructions
            i = 0
            while i < len(insts):
                ins = insts[i]
                if (
                    ins.sync_info is not None
                    and len(ins.sync_info.on_wait) > max_waits
                ):
                    waits = list(ins.sync_info.on_wait)
                    keep, extra = waits[:max_waits], waits[max_waits:]
                    ins.sync_info.on_wait = keep
                    for j, wchunk in enumerate(extra):
                        nop = mybir.InstNoOp(
                            name=f"{ins.name}-wsplit{j}",
                            sync_info=mybir.SyncInfo(on_wait=[wchunk], on_update=[]),
                            bass_nofuse=True,
                            engine=ins.engine,
                        )
                        nc.register_instruction(nop, overwrite=True)
                        insts.insert(i, nop)
                        i += 1
                i += 1


def _get_nc():
    global _NC
    if _NC is None:
        _NC = _build_nc()
        _split_ctrl_waits(_NC)
    return _NC


def run(enc_x, weight, bias, trace=False, **spmd_kwargs):
    """Run on 8 NeuronCores; returns (out [B, WINDOWS] fp32, BassKernelResults)."""
    from concourse.bass_utils import run_bass_kernel_spmd

    nc = _get_nc()
    xf = np.ascontiguousarray(np.asarray(enc_x), dtype=np.float32).reshape(
        NCORES, NWIN, K
    )
    wf = np.ascontiguousarray(np.asarray(weight), dtype=np.float32).reshape(K)
    bf = float(np.asarray(bias).reshape(-1)[0])
    in_maps = [{"x": xf[i], "w": wf} for i in range(NCORES)]
    res = run_bass_kernel_spmd(
        nc, in_maps, list(range(NCORES)), trace=trace, **spmd_kwargs
    )
    out = np.stack([res.results[i]["out"] for i in range(NCORES)], axis=0)
    out = out.astype(np.float32) + np.float32(bf)
    return out.reshape(B, WINDOWS), res


def kernel(enc_x, weight, bias, windows_nb=None):
    out, _ = run(enc_x, weight, bias)
    return out


# revision 5
# speedup vs baseline: 1.3613x; 1.1092x over previous
"""Trainium2 Bass kernel for im2col Conv2d dot-product (v4, bf16 hybrid):
out[b, n] = <enc_x[b, n, :], w_flat> + bias.

Data-parallel over batch: 8 batches per NeuronCore x 8 cores.
Per core x [401408, 49] fp32 (~78.7 MB); HBM roofline ~225 us/core.

Key moves (probe-measured rates per [128, 196*49] tile):
  - SWDGE cast-DMA loads x as bf16 (HBM still reads fp32 -> same 220us
    DMA floor, but halves SBUF traffic/footprint; the Pool-queue trigger
    costs only ~0.8us/tile and GpSimd does no other work -> no POOL-port
    contention with DVE).
  - multiply: DVE bf16 tensor_tensor runs in 2x mode (5.26us vs 10.1
    fp32); ~4.5 tile-equivalents go to the otherwise-idle ScalarE as 49
    strided per-k activation muls (0.72us/col, own SBUF ports).
  - segmented reduce: DVE tensor_reduce (10.1us, the only free-axis
    reducer).  DVE total ~177us, ACT ~175us, both under the DMA floor.
  - ScalarE-tile reduces are emitted 2 tiles late so the in-order DVE
    queue never stalls waiting on ACT's 35us multiply chain.
  - output DMAs ride the (otherwise idle) SP HWDGE ring; bias is added
    on the host (elementwise on the small gathered output).
bf16 rounding of x and w gives rel err ~2e-3, well inside the 2e-2 gate.
"""

from contextlib import ExitStack

import numpy as np

import concourse.bass as bass
import concourse.tile as tile
from concourse import mybir

B = 64
WINDOWS = 50176
K = 49
NCORES = 8
BPC = B // NCORES
NWIN = BPC * WINDOWS
P = 128

WBIG = 196
WSMALL = 49
TBIG = 15
TSMALL = 4
assert TBIG * P * WBIG + TSMALL * P * WSMALL == NWIN

# Tile emission order: (windows-per-partition, mult engine).  Small tiles
# lead (fast pipeline fill) and trail (short drain); the 4 ScalarE-mult
# tiles sit early-middle so their long 49-col chains never gate the tail.
TILES = [(WSMALL, "dve"), (WSMALL, "dve"),
         (WBIG, "dve"), (WBIG, "act"), (WBIG, "dve"), (WBIG, "dve"),
         (WBIG, "act"), (WBIG, "dve"), (WBIG, "dve"), (WBIG, "act"),
         (WBIG, "dve"), (WBIG, "dve"), (WBIG, "act"), (WBIG, "dve"),
         (WBIG, "dve"), (WBIG, "dve"), (WBIG, "dve"),
         (WSMALL, "dve"), (WSMALL, "dve")]
ACT_LAG = 2   # act-tile reduce/out deferred this many tiles

FP32 = mybir.dt.float32
BF16 = mybir.dt.bfloat16

_NC = None


def _build_nc():
    nc = bass.Bass(trn_type="TRN2", debug=False, num_devices=NCORES)

    x = nc.dram_tensor("x", [NWIN, K], FP32, kind="ExternalInput").ap()
    w = nc.dram_tensor("w", [K], FP32, kind="ExternalInput").ap()
    out = nc.dram_tensor("out", [NWIN], FP32, kind="ExternalOutput").ap()

    mult = mybir.AluOpType.mult
    add = mybir.AluOpType.add

    with tile.TileContext(nc) as tc, ExitStack() as ctx:
        consts = ctx.enter_context(tc.tile_pool(name="consts", bufs=1))
        xpool = ctx.enter_context(tc.tile_pool(name="x", bufs=6))
        opool = ctx.enter_context(tc.tile_pool(name="o", bufs=8))

        wb = consts.tile([P, K], FP32)
        nc.gpsimd.dma_start(
            out=wb[:],
            in_=bass.AP(tensor=w.tensor, offset=w.offset, ap=[[0, P]] + list(w.ap)),
        )
        wbb = consts.tile([P, K], BF16)
        nc.vector.tensor_copy(out=wbb[:], in_=wb[:])
        wb_ap = wb[:]
        wbb_ap = wbb[:]

        def w_bcast(wn):
            return bass.AP(
                tensor=wbb_ap.tensor, offset=wbb_ap.offset,
                ap=[wbb_ap.ap[0], [0, wn], wbb_ap.ap[1]],
            )

        deferred = []   # [(emit_after_tile_idx, fn), ...]

        def finish_tile(xt_ap, wn, win_base, name):
            xt3d = bass.AP(tensor=xt_ap.tensor, offset=xt_ap.offset,
                           ap=[xt_ap.ap[0], [K, wn], [1, K]])
            acc = opool.tile([P, wn], FP32, tag="acc", name=f"acc{name}")
            nc.vector.tensor_reduce(out=acc[:], in_=xt3d,
                                    axis=mybir.AxisListType.X, op=add)
            dst = bass.AP(
                tensor=out.tensor, offset=out.offset + win_base,
                ap=[[wn, P], [1, wn]],
            )
            nc.sync.dma_start(out=dst, in_=acc[:])

        def do_tile(idx, win_base, wn, mode, name):
            fd = wn * K
            xt = xpool.tile([P, fd], BF16, tag="xt", name=f"xt{name}")
            xt_ap = xt[:]
            src = bass.AP(
                tensor=x.tensor,
                offset=x.offset + win_base * K,
                ap=[[fd, P], [1, fd]],
            )
            nc.gpsimd.dma_start(out=xt_ap, in_=src)   # SWDGE cast fp32->bf16

            if mode == "dve":
                xt3d = bass.AP(tensor=xt_ap.tensor, offset=xt_ap.offset,
                               ap=[xt_ap.ap[0], [K, wn], [1, K]])
                nc.vector.tensor_tensor(out=xt3d, in0=xt3d,
                                        in1=w_bcast(wn), op=mult)
                finish_tile(xt_ap, wn, win_base, name)
            else:
                for kk in range(K):
                    col = bass.AP(
                        tensor=xt_ap.tensor, offset=xt_ap.offset + kk,
                        ap=[xt_ap.ap[0], [K, wn]],
                    )
                    nc.scalar.activation(
                        out=col, in_=col,
                        func=mybir.ActivationFunctionType.Identity,
                        scale=wb_ap[:, kk:kk + 1],
                    )
                deferred.append(
                    (idx + ACT_LAG,
                     lambda a=xt_ap, b=wn, c=win_base, d=name: finish_tile(a, b, c, d))
                )

        def flush_deferred(now):
            while deferred and deferred[0][0] <= now:
                deferred.pop(0)[1]()

        base = 0
        for t, (wn, mode) in enumerate(TILES):
            do_tile(t, base, wn, mode, f"t{t}")
            flush_deferred(t)
            base += P * wn
        flush_deferred(10**9)
        assert base == NWIN

    return nc


def _split_ctrl_waits(nc, max_waits=1):
    """Work around a walrus codegen limit on this build: instructions accept
    only one sync-wait command. Hoist extra waits onto dedicated no-op
    instructions inserted just before, preserving per-engine order."""
    from concourse import mybir

    for f in nc.m.functions:
        for blk in f.blocks:
            insts = blk.instructions
            i = 0
            while i < len(insts):
                ins = insts[i]
                if (
                    ins.sync_info is not None
                    and len(ins.sync_info.on_wait) > max_waits
                ):
                    waits = list(ins.sync_info.on_wait)
                    keep, extra = waits[:max_waits], waits[max_waits:]
                    ins.sync_info.on_wait = keep
                    for j, wchunk in enumerate(extra):
                        nop = mybir.InstNoOp(
                            name=f"{ins.name}-wsplit{j}",
                            sync_info=mybir.SyncInfo(on_wait=[wchunk], on_update=[]),
                            bass_nofuse=True,
                            engine=ins.engine,
                        )
                        nc.register_instruction(nop, overwrite=True)
                        insts.insert(i, nop)
                        i += 1
                i += 1


def _get_nc():
    global _NC
    if _NC is None:
        _NC = _build_nc()
        _split_ctrl_waits(_NC)
    return _NC


def run(enc_x, weight, bias, trace=False, **spmd_kwargs):
    """Run on 8 NeuronCores; returns (out [B, WINDOWS] fp32, BassKernelResults)."""
    from concourse.bass_utils import run_bass_kernel_spmd

    nc = _get_nc()
    xf = np.ascontiguousarray(np.asarray(enc_x), dtype=np.float32).reshape(
        NCORES, NWIN, K
    )
    wf = np.ascontiguousarray(np.asarray(weight), dtype=np.float32).reshape(K)
    bf = float(np.asarray(bias).reshape(-1)[0])
    in_maps = [{"x": xf[i], "w": wf} for i in range(NCORES)]
    res = run_bass_kernel_spmd(
        nc, in_maps, list(range(NCORES)), trace=trace, **spmd_kwargs
    )
    out = np.stack([res.results[i]["out"] for i in range(NCORES)], axis=0)
    out = out.astype(np.float32) + np.float32(bf)
    return out.reshape(B, WINDOWS), res


def kernel(enc_x, weight, bias, windows_nb=None):
    out, _ = run(enc_x, weight, bias)
    return out


# revision 6
# speedup vs baseline: 1.3629x; 1.0012x over previous
"""Trainium2 Bass kernel for im2col Conv2d dot-product (v4, bf16 hybrid):
out[b, n] = <enc_x[b, n, :], w_flat> + bias.

Data-parallel over batch: 8 batches per NeuronCore x 8 cores.
Per core x [401408, 49] fp32 (~78.7 MB); HBM roofline ~225 us/core.

Key moves (probe-measured rates per [128, 196*49] tile):
  - SWDGE cast-DMA loads x as bf16 (HBM still reads fp32 -> same 220us
    DMA floor, but halves SBUF traffic/footprint; the Pool-queue trigger
    costs only ~0.8us/tile and GpSimd does no other work -> no POOL-port
    contention with DVE).
  - multiply: DVE bf16 tensor_tensor runs in 2x mode (5.26us vs 10.1
    fp32); ~4.5 tile-equivalents go to the otherwise-idle ScalarE as 49
    strided per-k activation muls (0.72us/col, own SBUF ports).
  - segmented reduce: DVE tensor_reduce (10.1us, the only free-axis
    reducer).  DVE total ~177us, ACT ~175us, both under the DMA floor.
  - ScalarE-tile reduces are emitted 2 tiles late so the in-order DVE
    queue never stalls waiting on ACT's 35us multiply chain.
  - output DMAs ride the (otherwise idle) SP HWDGE ring; bias is added
    on the host (elementwise on the small gathered output).
bf16 rounding of x and w gives rel err ~2e-3, well inside the 2e-2 gate.
"""

from contextlib import ExitStack

import numpy as np

import concourse.bass as bass
import concourse.tile as tile
from concourse import mybir

B = 64
WINDOWS = 50176
K = 49
NCORES = 8
BPC = B // NCORES
NWIN = BPC * WINDOWS
P = 128

WBIG = 196
WSMALL = 49
TBIG = 15
TSMALL = 4
assert TBIG * P * WBIG + TSMALL * P * WSMALL == NWIN

# Tile emission order: (windows-per-partition, mult engine).  Small tiles
# lead (fast pipeline fill) and trail (short drain); the 4 ScalarE-mult
# tiles sit early-middle so their long 49-col chains never gate the tail.
TILES = [(WSMALL, "dve"), (WSMALL, "dve"),
         (WBIG, "dve"), (WBIG, "act"), (WBIG, "dve"), (WBIG, "dve"),
         (WBIG, "act"), (WBIG, "dve"), (WBIG, "dve"), (WBIG, "act"),
         (WBIG, "dve"), (WBIG, "dve"), (WBIG, "act"), (WBIG, "dve"),
         (WBIG, "dve"), (WBIG, "dve"), (WBIG, "dve"),
         (WSMALL, "dve"), (WSMALL, "dve")]
ACT_LAG = 3   # act-tile reduce/out deferred this many tiles

FP32 = mybir.dt.float32
BF16 = mybir.dt.bfloat16

_NC = None


def _build_nc():
    nc = bass.Bass(trn_type="TRN2", debug=False, num_devices=NCORES)

    x = nc.dram_tensor("x", [NWIN, K], FP32, kind="ExternalInput").ap()
    w = nc.dram_tensor("w", [K], FP32, kind="ExternalInput").ap()
    out = nc.dram_tensor("out", [NWIN], FP32, kind="ExternalOutput").ap()

    mult = mybir.AluOpType.mult
    add = mybir.AluOpType.add

    with tile.TileContext(nc) as tc, ExitStack() as ctx:
        consts = ctx.enter_context(tc.tile_pool(name="consts", bufs=1))
        xpool = ctx.enter_context(tc.tile_pool(name="x", bufs=8))
        opool = ctx.enter_context(tc.tile_pool(name="o", bufs=8))

        wb = consts.tile([P, K], FP32)
        wbb = consts.tile([P, K], BF16)
        wb_ap = wb[:]
        wbb_ap = wbb[:]

        def load_weights():
            # emitted after the first tile's DMA trigger so the input
            # stream starts immediately at t=0
            nc.gpsimd.dma_start(
                out=wb_ap,
                in_=bass.AP(tensor=w.tensor, offset=w.offset,
                            ap=[[0, P]] + list(w.ap)),
            )
            nc.vector.tensor_copy(out=wbb_ap, in_=wb_ap)

        def w_bcast(wn):
            return bass.AP(
                tensor=wbb_ap.tensor, offset=wbb_ap.offset,
                ap=[wbb_ap.ap[0], [0, wn], wbb_ap.ap[1]],
            )

        deferred = []   # [(emit_after_tile_idx, fn), ...]

        def finish_tile(xt_ap, wn, win_base, name):
            xt3d = bass.AP(tensor=xt_ap.tensor, offset=xt_ap.offset,
                           ap=[xt_ap.ap[0], [K, wn], [1, K]])
            acc = opool.tile([P, wn], FP32, tag="acc", name=f"acc{name}")
            nc.vector.tensor_reduce(out=acc[:], in_=xt3d,
                                    axis=mybir.AxisListType.X, op=add)
            dst = bass.AP(
                tensor=out.tensor, offset=out.offset + win_base,
                ap=[[wn, P], [1, wn]],
            )
            nc.sync.dma_start(out=dst, in_=acc[:])

        def do_tile(idx, win_base, wn, mode, name, defer_compute=False):
            fd = wn * K
            xt = xpool.tile([P, fd], BF16, tag="xt", name=f"xt{name}")
            xt_ap = xt[:]
            src = bass.AP(
                tensor=x.tensor,
                offset=x.offset + win_base * K,
                ap=[[fd, P], [1, fd]],
            )
            nc.gpsimd.dma_start(out=xt_ap, in_=src)   # SWDGE cast fp32->bf16
            if defer_compute:
                return lambda: _tile_compute(idx, xt_ap, wn, win_base, mode, name)
            _tile_compute(idx, xt_ap, wn, win_base, mode, name)

        def _tile_compute(idx, xt_ap, wn, win_base, mode, name):
            if mode == "dve":
                xt3d = bass.AP(tensor=xt_ap.tensor, offset=xt_ap.offset,
                               ap=[xt_ap.ap[0], [K, wn], [1, K]])
                nc.vector.tensor_tensor(out=xt3d, in0=xt3d,
                                        in1=w_bcast(wn), op=mult)
                finish_tile(xt_ap, wn, win_base, name)
            else:
                for kk in range(K):
                    col = bass.AP(
                        tensor=xt_ap.tensor, offset=xt_ap.offset + kk,
                        ap=[xt_ap.ap[0], [K, wn]],
                    )
                    nc.scalar.activation(
                        out=col, in_=col,
                        func=mybir.ActivationFunctionType.Identity,
                        scale=wb_ap[:, kk:kk + 1],
                    )
                deferred.append(
                    (idx + ACT_LAG,
                     lambda a=xt_ap, b=wn, c=win_base, d=name: finish_tile(a, b, c, d))
                )

        def flush_deferred(now):
            while deferred and deferred[0][0] <= now:
                deferred.pop(0)[1]()

        base = 0
        for t, (wn, mode) in enumerate(TILES):
            if t == 0:
                compute0 = do_tile(t, base, wn, mode, f"t{t}",
                                   defer_compute=True)
                load_weights()
                compute0()
            else:
                do_tile(t, base, wn, mode, f"t{t}")
            flush_deferred(t)
            base += P * wn
        flush_deferred(10**9)
        assert base == NWIN

    return nc


def _split_ctrl_waits(nc, max_waits=1):
    """Work around a walrus codegen limit on this build: instructions accept
    only one sync-wait command. Hoist extra waits onto dedicated no-op
    instructions inserted just before, preserving per-engine order."""
    from concourse import mybir

    for f in nc.m.functions:
        for blk in f.blocks:
            insts = blk.instructions
            i = 0
            while i < len(insts):
                ins = insts[i]
                if (
                    ins.sync_info is not None
                    and len(ins.sync_info.on_wait) > max_waits
                ):
                    waits = list(ins.sync_info.on_wait)
                    keep, extra = waits[:max_waits], waits[max_waits:]
                    ins.sync_info.on_wait = keep
                    for j, wchunk in enumerate(extra):
                        nop = mybir.InstNoOp(
                            name=f"{ins.name}-wsplit{j}",
                            sync_info=mybir.SyncInfo(on_wait=[wchunk], on_update=[]),
                            bass_nofuse=True,
                            engine=ins.engine,
                        )
                        nc.register_instruction(nop, overwrite=True)
                        insts.insert(i, nop)
                        i += 1
                i += 1


def _get_nc():
    global _NC
    if _NC is None:
        _NC = _build_nc()
        _split_ctrl_waits(_NC)
    return _NC


def run(enc_x, weight, bias, trace=False, **spmd_kwargs):
    """Run on 8 NeuronCores; returns (out [B, WINDOWS] fp32, BassKernelResults)."""
    from concourse.bass_utils import run_bass_kernel_spmd

    nc = _get_nc()
    xf = np.ascontiguousarray(np.asarray(enc_x), dtype=np.float32).reshape(
        NCORES, NWIN, K
    )
    wf = np.ascontiguousarray(np.asarray(weight), dtype=np.float32).reshape(K)
    bf = float(np.asarray(bias).reshape(-1)[0])
    in_maps = [{"x": xf[i], "w": wf} for i in range(NCORES)]
    res = run_bass_kernel_spmd(
        nc, in_maps, list(range(NCORES)), trace=trace, **spmd_kwargs
    )
    out = np.stack([res.results[i]["out"] for i in range(NCORES)], axis=0)
    out = out.astype(np.float32) + np.float32(bf)
    return out.reshape(B, WINDOWS), res


def kernel(enc_x, weight, bias, windows_nb=None):
    out, _ = run(enc_x, weight, bias)
    return out
